# revision 15
# baseline (speedup 1.0000x reference)
"""Single-head causal attention (B=8, T=2048, C=768, H=64) on 8 TRN2 NeuronCores.

Strategy: data-parallel over batch (1 batch element per core, SPMD, no
collectives). Host pre-transposes x to [C, T] and casts inputs to bf16 so the
device kernel needs no on-chip transposes of x.

Per-core dataflow (all matmuls bf16 with f32 PSUM accumulation):
  1. qkT[128, T] = [Wq|Wk].T @ xT           (qT rows 0:64, kT rows 64:128)
     kT shifted to partitions 0:63 and qT to partitions 64:127 via SBUF->SBUF
     DMA so score matmuls can be packed onto both PE row-halves.
  2. v[s,64] per 128-row s-block: xT-block stationary, Wv moving. Stored into
     v_aug[128, 16, 65] whose last column is 1.0 (softmax denominator trick).
  3. For each 512-wide t-group g: for s-blocks j<=4g+3:
       scoresT[s,t] = kT_j.T @ qT   (PSUM; even/odd j concurrently on PE
       row-halves 0/1 via tile_position), exp(0.125*x) on ScalarE -> bf16 P.T;
       diagonal blocks masked by upper-triangular 0/1 constant (GPSIMD).
       outT_aug[65, 512] += [v_j|1].T @ P.T   (PSUM accumulate over j)
     Row 64 of outT_aug = sum of exp = softmax denominator. No max-subtraction
     is needed: scores are ~N(0,1) for this problem (verified; |s|<20 safe).
  4. Copy outT_aug to bf16, transpose 128-col blocks on PE (plain bf16 matmul
     against identity), reciprocal of col 64, scale, DMA out f32 [T, 64].
"""
import sys

for _p in ("/opt/trn_rl_repo",):
    if _p not in sys.path:
        sys.path.insert(0, _p)

import numpy as np
import ml_dtypes

import concourse.bass as bass
import concourse.tile as tile
from concourse import bacc, mybir
from concourse.bass_utils import run_bass_kernel_spmd
from concourse.masks import make_identity, make_upper_triangular

F32 = mybir.dt.float32
BF16 = mybir.dt.bfloat16

B, T, C, H = 8, 2048, 768, 64
CC = C // 128          # 6 contraction chunks
NG = T // 512          # 4 t-groups
SCALE = float(H) ** -0.5


def _chunk_groups(g):
    """s-block chunk grouping for t-group g: (even, odd) pairs of full-width
    (512) chunks, then singles (the odd full one + 3 diagonal partials)."""
    full = list(range(4 * g + 1))
    groups = [full[i:i + 2] for i in range(0, len(full) - 1, 2)]
    if len(full) % 2 == 1:
        groups.append([full[-1]])
    groups += [[4 * g + 1], [4 * g + 2], [4 * g + 3]]
    return groups


def _build():
    nc = bacc.Bacc("TRN2", target_bir_lowering=False, debug=False, num_devices=8)
    xT = nc.declare_dram_parameter("xT", [C, T], BF16, isOutput=False)
    wqk = nc.declare_dram_parameter("wqk", [C, 128], BF16, isOutput=False)
    wv = nc.declare_dram_parameter("wv", [C, H], BF16, isOutput=False)
    out = nc.declare_dram_parameter("out", [T, H], F32, isOutput=True)

    xT_r = xT.rearrange("(n p) t -> p n t", p=128)    # [128, CC, T]
    wqk_r = wqk.rearrange("(n p) m -> p n m", p=128)  # [128, CC, 128]
    wv_r = wv.rearrange("(n p) m -> p n m", p=128)    # [128, CC, H]
    out_r = out.rearrange("(n p) h -> p n h", p=128)  # [128, 16, H]

    with tile.TileContext(nc) as tc:
        with (
            tc.tile_pool(name="const", bufs=1) as const,
            tc.tile_pool(name="big", bufs=1) as big,
            tc.tile_pool(name="pt", bufs=4) as ptp,
            tc.tile_pool(name="ev", bufs=2) as ev,
            tc.tile_pool(name="ps_s", bufs=2, space="PSUM") as ps_s,
            tc.tile_pool(name="ps_m", bufs=2, space="PSUM") as ps_m,
            tc.tile_pool(name="ps_o", bufs=2, space="PSUM") as ps_o,
        ):
            # constants
            tri = const.tile([128, 128], BF16)
            make_upper_triangular(nc, tri, val=1.0, diag=True)
            ident = const.tile([128, 128], BF16)
            make_identity(nc, ident)
            # warm the ACT exp table load while DMAs run
            warm = const.tile([128, 1], F32)
            nc.vector.memset(warm, 0.0)
            nc.scalar.activation(warm, warm, mybir.ActivationFunctionType.Exp)

            # weights
            w_qk = const.tile([128, CC, 128], BF16)
            nc.sync.dma_start(out=w_qk[:], in_=wqk_r)
            w_v = const.tile([128, CC, H], BF16)
            nc.sync.dma_start(out=w_v[:], in_=wv_r)

            # x (two DMAs per t-group so compute can start early)
            x_sb = big.tile([128, CC, T], BF16)
            for g in range(NG):
                gs = slice(512 * g, 512 * (g + 1))
                nc.sync.dma_start(out=x_sb[:, 0:3, gs], in_=xT_r[:, 0:3, gs])
                nc.sync.dma_start(out=x_sb[:, 3:6, gs], in_=xT_r[:, 3:6, gs])

            # persistent attention operands
            qk_sb = big.tile([128, T], BF16)   # rows 0:64 qT, rows 64:128 kT
            k_st = big.tile([64, T], BF16)     # kT shifted to partitions 0:63
            q_hi = big.tile([128, T], BF16)    # qT at partitions 64:127
            v_aug = big.tile([128, 16, H + 1], BF16)
            nc.vector.memset(v_aug[:, :, H:H + 1], 1.0)

            def evict(g, p_out):
                """transpose outT_aug[65, 512] via bf16 PE matmuls, normalize,
                store t-group g"""
                oT = ev.tile([H + 1, 512], BF16, tag="oT")
                nc.vector.tensor_copy(oT[:], p_out[:])
                p_tr = ps_m.tile([128, 4, H + 1], F32, tag="psm")
                for i in range(4):
                    nc.tensor.matmul(
                        p_tr[:, i, :], lhsT=oT[:, 128 * i:128 * (i + 1)],
                        rhs=ident[0:H + 1, 0:H + 1], start=True, stop=True,
                    )
                rec = ev.tile([128, 4, 1], F32, tag="rec")
                nc.vector.reciprocal(rec[:], p_tr[:, :, H:H + 1])
                o_sb = ev.tile([128, 4, H], F32, tag="osb")
                for i in range(4):
                    nc.vector.tensor_scalar_mul(
                        o_sb[:, i, :], p_tr[:, i, 0:H], rec[:, i, :]
                    )
                nc.sync.dma_start(out=out_r[:, 4 * g:4 * g + 4, :], in_=o_sb[:])

            pending_evict = None
            for g in range(NG):
                gs = slice(512 * g, 512 * (g + 1))
                # ---- qk projection for this t-group ----
                p_qk = ps_m.tile([128, 512], F32, tag="psm")
                for cc in range(CC):
                    nc.tensor.matmul(
                        p_qk[:], lhsT=w_qk[:, cc, :], rhs=x_sb[:, cc, gs],
                        start=(cc == 0), stop=(cc == CC - 1),
                    )
                nc.vector.tensor_copy(qk_sb[:, gs], p_qk[:])
                # partition shifts on the SWDGE queue: keeps them off the Sync
                # FIFO so they don't serialize behind the big x loads
                nc.gpsimd.dma_start(out=k_st[:, gs], in_=qk_sb[64:128, gs])
                nc.gpsimd.dma_start(out=q_hi[64:128, gs], in_=qk_sb[0:64, gs])

                # ---- v projection for s-blocks 4g..4g+3 ----
                p_v = ps_m.tile([128, 4, H], F32, tag="psm")
                for i in range(4):
                    ss = slice(128 * (4 * g + i), 128 * (4 * g + i + 1))
                    for cc in range(CC):
                        nc.tensor.matmul(
                            p_v[:, i, :], lhsT=x_sb[:, cc, ss], rhs=w_v[:, cc, :],
                            start=(cc == 0), stop=(cc == CC - 1),
                        )
                nc.vector.tensor_copy(v_aug[:, 4 * g:4 * g + 4, 0:H], p_v[:])

                # ---- attention: t-chunk g over s-blocks 0..4g+3 ----
                p_out = ps_o.tile([H + 1, 512], F32)
                n_j = 4 * g + 4
                pending = None  # [(j, chunk offset in group, width, pt, idx)]
                for grp in _chunk_groups(g):
                    widths = [512 * (g + 1) - max(128 * j, 512 * g) for j in grp]
                    w0 = widths[0]
                    p_sc = ps_s.tile([128, 2, 512], F32, tag="pss")
                    for idx, j in enumerate(grp):
                        t_lo = max(128 * j, 512 * g)
                        jb = slice(128 * j, 128 * (j + 1))
                        tsl = slice(t_lo, 512 * (g + 1))
                        if j % 2 == 0:  # PE row-half 0
                            nc.tensor.matmul(
                                p_sc[:, idx, 0:widths[idx]],
                                lhsT=k_st[:, jb], rhs=qk_sb[0:64, tsl],
                                start=True, stop=True, tile_position=(0, 0),
                            )
                        else:           # PE row-half 1, runs concurrently
                            nc.tensor.matmul(
                                p_sc[:, idx, 0:widths[idx]],
                                lhsT=qk_sb[64:128, jb], rhs=q_hi[64:128, tsl],
                                start=True, stop=True, tile_position=(64, 0),
                            )
                    pt = ptp.tile([128, 2, 512], BF16, tag="pt")
                    if len(grp) == 2:
                        nc.scalar.activation(
                            pt[:], p_sc[:],
                            mybir.ActivationFunctionType.Exp, scale=SCALE,
                        )
                    else:
                        nc.scalar.activation(
                            pt[:, 0, 0:w0], p_sc[:, 0, 0:w0],
                            mybir.ActivationFunctionType.Exp, scale=SCALE,
                        )
                    for idx, j in enumerate(grp):
                        if 128 * j >= 512 * g:  # diagonal block at chunk offset 0
                            nc.gpsimd.tensor_mul(
                                pt[:, idx, 0:128], pt[:, idx, 0:128], tri[:]
                            )
                    if pending is not None:
                        for (pj, poff, pw, ppt, pidx) in pending:
                            nc.tensor.matmul(
                                p_out[:, poff:poff + pw],
                                lhsT=v_aug[:, pj, :],
                                rhs=ppt[:, pidx, 0:pw],
                                start=(pj == 0), stop=False,
                            )
                    pending = [
                        (j, max(128 * j, 512 * g) - 512 * g, widths[idx], pt, idx)
                        for idx, j in enumerate(grp)
                    ]
                for (pj, poff, pw, ppt, pidx) in pending:
                    nc.tensor.matmul(
                        p_out[:, poff:poff + pw],
                        lhsT=v_aug[:, pj, :],
                        rhs=ppt[:, pidx, 0:pw],
                        start=(pj == 0), stop=(pj == n_j - 1),
                    )

                # evict the PREVIOUS group now that this group's matmuls are
                # emitted — gives PE work to overlap the eviction chain
                if pending_evict is not None:
                    evict(*pending_evict)
                pending_evict = (g, p_out)
            evict(*pending_evict)

    nc.compile()
    return nc


_NC = None


def _get_nc():
    global _NC
    if _NC is None:
        _NC = _build()
    return _NC


def _prep_inputs(x, Wq, Wk, Wv):
    bf = ml_dtypes.bfloat16
    xT = np.ascontiguousarray(np.transpose(x, (0, 2, 1))).astype(bf)
    wqk = np.ascontiguousarray(np.concatenate([Wq, Wk], axis=1)).astype(bf)
    wv = np.ascontiguousarray(Wv).astype(bf)
    return [{"xT": xT[b], "wqk": wqk, "wv": wv} for b in range(B)]


def run_cores(x, Wq, Wk, Wv, trace=False):
    nc = _get_nc()
    res = run_bass_kernel_spmd(
        nc, _prep_inputs(x, Wq, Wk, Wv), core_ids=list(range(B)), trace=trace
    )
    out = np.stack([res.results[b]["out"] for b in range(B)], axis=0)
    return out.astype(np.float32), res


def kernel(x, Wq, Wk, Wv):
    out, _ = run_cores(np.asarray(x), np.asarray(Wq), np.asarray(Wk), np.asarray(Wv))
    return out


# revision 16
# speedup vs baseline: 1.0850x; 1.0850x over previous
"""Single-head causal attention (B=8, T=2048, C=768, H=64) on 8 TRN2 NeuronCores.

Strategy: data-parallel over batch (1 batch element per core, SPMD, no
collectives). Host pre-transposes x to [C, T] and casts inputs to bf16 so the
device kernel needs no on-chip transposes of x.

Per-core dataflow (all matmuls bf16 with f32 PSUM accumulation):
  1. qkT[128, T] = [Wq|Wk].T @ xT           (qT rows 0:64, kT rows 64:128)
     kT shifted to partitions 0:63 and qT to partitions 64:127 via SBUF->SBUF
     DMA so score matmuls can be packed onto both PE row-halves.
  2. v[s,64] per 128-row s-block: xT-block stationary, Wv moving. Stored into
     v_aug[128, 16, 65] whose last column is 1.0 (softmax denominator trick).
  3. For each 512-wide t-group g: for s-blocks j<=4g+3:
       scoresT[s,t] = kT_j.T @ qT   (PSUM; even/odd j concurrently on PE
       row-halves 0/1 via tile_position), exp(0.125*x) on ScalarE -> bf16 P.T;
       diagonal blocks masked by upper-triangular 0/1 constant (GPSIMD).
       outT_aug[65, 512] += [v_j|1].T @ P.T   (PSUM accumulate over j)
     Row 64 of outT_aug = sum of exp = softmax denominator. No max-subtraction
     is needed: scores are ~N(0,1) for this problem (verified; |s|<20 safe).
  4. Copy outT_aug to bf16, transpose 128-col blocks on PE (plain bf16 matmul
     against identity), reciprocal of col 64, scale, DMA out f32 [T, 64].
"""
import sys

for _p in ("/opt/trn_rl_repo",):
    if _p not in sys.path:
        sys.path.insert(0, _p)

import numpy as np
import ml_dtypes

import concourse.bass as bass
import concourse.tile as tile
from concourse import bacc, mybir
from concourse.bass_utils import run_bass_kernel_spmd
from concourse.masks import make_identity, make_upper_triangular

F32 = mybir.dt.float32
BF16 = mybir.dt.bfloat16

B, T, C, H = 8, 2048, 768, 64
CC = C // 128          # 6 contraction chunks
NG = T // 512          # 4 t-groups
SCALE = float(H) ** -0.5


def _chunk_groups(g):
    """s-block chunk grouping for t-group g: (even, odd) pairs of full-width
    (512) chunks, then singles (the odd full one + 3 diagonal partials)."""
    full = list(range(4 * g + 1))
    groups = [full[i:i + 2] for i in range(0, len(full) - 1, 2)]
    if len(full) % 2 == 1:
        groups.append([full[-1]])
    groups += [[4 * g + 1], [4 * g + 2], [4 * g + 3]]
    return groups


def _build():
    nc = bacc.Bacc("TRN2", target_bir_lowering=False, debug=False, num_devices=8)
    xT = nc.declare_dram_parameter("xT", [C, T], BF16, isOutput=False)
    wqk = nc.declare_dram_parameter("wqk", [C, 128], BF16, isOutput=False)
    wkq = nc.declare_dram_parameter("wkq", [C, 128], BF16, isOutput=False)
    wv = nc.declare_dram_parameter("wv", [C, H], BF16, isOutput=False)
    out = nc.declare_dram_parameter("out", [T, H], F32, isOutput=True)

    xT_r = xT.rearrange("(n p) t -> p n t", p=128)    # [128, CC, T]
    wqk_r = wqk.rearrange("(n p) m -> p n m", p=128)  # [128, CC, 128]
    wkq_r = wkq.rearrange("(n p) m -> p n m", p=128)  # [128, CC, 128]
    wv_r = wv.rearrange("(n p) m -> p n m", p=128)    # [128, CC, H]
    out_r = out.rearrange("(n p) h -> p n h", p=128)  # [128, 16, H]

    with tile.TileContext(nc) as tc:
        with (
            tc.tile_pool(name="const", bufs=1) as const,
            tc.tile_pool(name="big", bufs=1) as big,
            tc.tile_pool(name="pt", bufs=6) as ptp,
            tc.tile_pool(name="ev", bufs=2) as ev,
            tc.tile_pool(name="ps_s", bufs=4, space="PSUM") as ps_s,
            tc.tile_pool(name="ps_m", bufs=2, space="PSUM") as ps_m,
            tc.tile_pool(name="ps_o", bufs=2, space="PSUM") as ps_o,
        ):
            # constants
            tri = const.tile([128, 128], BF16)
            make_upper_triangular(nc, tri, val=1.0, diag=True)
            ident = const.tile([128, 128], BF16)
            make_identity(nc, ident)
            # warm the ACT exp table load while DMAs run
            warm = const.tile([128, 1], F32)
            nc.vector.memset(warm, 0.0)
            nc.scalar.activation(warm, warm, mybir.ActivationFunctionType.Exp)

            # weights
            w_qk = const.tile([128, CC, 128], BF16)
            nc.sync.dma_start(out=w_qk[:], in_=wqk_r)
            w_kq = const.tile([128, CC, 128], BF16)
            nc.sync.dma_start(out=w_kq[:], in_=wkq_r)
            w_v = const.tile([128, CC, H], BF16)
            nc.sync.dma_start(out=w_v[:], in_=wv_r)

            # x (two DMAs per t-group so compute can start early)
            x_sb = big.tile([128, CC, T], BF16)
            for g in range(NG):
                gs = slice(512 * g, 512 * (g + 1))
                nc.sync.dma_start(out=x_sb[:, 0:3, gs], in_=xT_r[:, 0:3, gs])
                nc.sync.dma_start(out=x_sb[:, 3:6, gs], in_=xT_r[:, 3:6, gs])

            # persistent attention operands
            qk_sb = big.tile([128, T], BF16)   # rows 0:64 qT, rows 64:128 kT
            kq_sb = big.tile([128, T], BF16)   # rows 0:64 kT, rows 64:128 qT
            v_aug = big.tile([128, 16, H + 1], BF16)
            nc.vector.memset(v_aug[:, :, H:H + 1], 1.0)

            def evict(g, p_out):
                """transpose outT_aug[65, 512] via bf16 PE matmuls, normalize,
                store t-group g"""
                oT = ev.tile([H + 1, 512], BF16, tag="oT")
                nc.vector.tensor_copy(oT[:], p_out[:])
                p_tr = ps_m.tile([128, 4, H + 1], F32, tag="psm")
                for i in range(4):
                    nc.tensor.matmul(
                        p_tr[:, i, :], lhsT=oT[:, 128 * i:128 * (i + 1)],
                        rhs=ident[0:H + 1, 0:H + 1], start=True, stop=True,
                    )
                rec = ev.tile([128, 4, 1], F32, tag="rec")
                nc.vector.reciprocal(rec[:], p_tr[:, :, H:H + 1])
                o_sb = ev.tile([128, 4, H], F32, tag="osb")
                for i in range(4):
                    nc.vector.tensor_scalar_mul(
                        o_sb[:, i, :], p_tr[:, i, 0:H], rec[:, i, :]
                    )
                nc.sync.dma_start(out=out_r[:, 4 * g:4 * g + 4, :], in_=o_sb[:])

            def proj(g):
                """projections for t-group g: qk pass, kq pass (swapped halves
                instead of partition-shift DMAs), and v for s-blocks 4g..4g+3"""
                gs = slice(512 * g, 512 * (g + 1))
                p_qk = ps_m.tile([128, 512], F32, tag="psm")
                for cc in range(CC):
                    nc.tensor.matmul(
                        p_qk[:], lhsT=w_qk[:, cc, 0:128], rhs=x_sb[:, cc, gs],
                        start=(cc == 0), stop=(cc == CC - 1),
                    )
                nc.vector.tensor_copy(qk_sb[:, gs], p_qk[:])
                p_kq = ps_m.tile([128, 512], F32, tag="psm")
                for cc in range(CC):
                    nc.tensor.matmul(
                        p_kq[:], lhsT=w_kq[:, cc, 0:128], rhs=x_sb[:, cc, gs],
                        start=(cc == 0), stop=(cc == CC - 1),
                    )
                nc.vector.tensor_copy(kq_sb[:, gs], p_kq[:])
                p_v = ps_m.tile([128, 4, H], F32, tag="psm")
                for i in range(4):
                    ss = slice(128 * (4 * g + i), 128 * (4 * g + i + 1))
                    for cc in range(CC):
                        nc.tensor.matmul(
                            p_v[:, i, :], lhsT=x_sb[:, cc, ss], rhs=w_v[:, cc, :],
                            start=(cc == 0), stop=(cc == CC - 1),
                        )
                nc.vector.tensor_copy(v_aug[:, 4 * g:4 * g + 4, 0:H], p_v[:])

            pending_evict = None
            proj(0)
            for g in range(NG):
                gs = slice(512 * g, 512 * (g + 1))
                # ---- attention: t-chunk g over s-blocks 0..4g+3 ----
                p_out = ps_o.tile([H + 1, 512], F32)
                n_j = 4 * g + 4
                pending = None  # (j, chunk offset in group, width, pt)
                for j in range(n_j):
                    t_lo = max(128 * j, 512 * g)
                    w = 512 * (g + 1) - t_lo
                    jb = slice(128 * j, 128 * (j + 1))
                    tsl = slice(t_lo, 512 * (g + 1))
                    p_sc = ps_s.tile([128, 512], F32, tag="pss")
                    if j % 2 == 0:  # PE row-half 0
                        nc.tensor.matmul(
                            p_sc[:, 0:w], lhsT=kq_sb[0:64, jb],
                            rhs=qk_sb[0:64, tsl],
                            start=True, stop=True, tile_position=(0, 0),
                        )
                    else:           # PE row-half 1
                        nc.tensor.matmul(
                            p_sc[:, 0:w], lhsT=qk_sb[64:128, jb],
                            rhs=kq_sb[64:128, tsl],
                            start=True, stop=True, tile_position=(64, 0),
                        )
                    pt = ptp.tile([128, 512], BF16, tag="pt")
                    nc.scalar.activation(
                        pt[:, 0:w], p_sc[:, 0:w],
                        mybir.ActivationFunctionType.Exp, scale=SCALE,
                    )
                    if 128 * j >= 512 * g:  # diagonal block at chunk offset 0
                        nc.gpsimd.tensor_mul(
                            pt[:, 0:128], pt[:, 0:128], tri[:]
                        )
                    if pending is not None:
                        (pj, poff, pw, ppt) = pending
                        nc.tensor.matmul(
                            p_out[:, poff:poff + pw],
                            lhsT=v_aug[:, pj, :], rhs=ppt[:, 0:pw],
                            start=(pj == 0), stop=False,
                        )
                    pending = (j, t_lo - 512 * g, w, pt)
                    if j == 1 and g + 1 < NG:
                        # feed PE the next group's projections while this
                        # group's exp-paced attention runs
                        proj(g + 1)
                (pj, poff, pw, ppt) = pending
                nc.tensor.matmul(
                    p_out[:, poff:poff + pw],
                    lhsT=v_aug[:, pj, :], rhs=ppt[:, 0:pw],
                    start=(pj == 0), stop=(pj == n_j - 1),
                )

                # evict the PREVIOUS group now that this group's matmuls are
                # emitted — gives PE work to overlap the eviction chain
                if pending_evict is not None:
                    evict(*pending_evict)
                pending_evict = (g, p_out)
            evict(*pending_evict)

    nc.compile()
    return nc


_NC = None


def _get_nc():
    global _NC
    if _NC is None:
        _NC = _build()
    return _NC


def _prep_inputs(x, Wq, Wk, Wv):
    bf = ml_dtypes.bfloat16
    xT = np.ascontiguousarray(np.transpose(x, (0, 2, 1))).astype(bf)
    wqk = np.ascontiguousarray(np.concatenate([Wq, Wk], axis=1)).astype(bf)
    wkq = np.ascontiguousarray(np.concatenate([Wk, Wq], axis=1)).astype(bf)
    wv = np.ascontiguousarray(Wv).astype(bf)
    return [{"xT": xT[b], "wqk": wqk, "wkq": wkq, "wv": wv} for b in range(B)]


def run_cores(x, Wq, Wk, Wv, trace=False):
    nc = _get_nc()
    res = run_bass_kernel_spmd(
        nc, _prep_inputs(x, Wq, Wk, Wv), core_ids=list(range(B)), trace=trace
    )
    out = np.stack([res.results[b]["out"] for b in range(B)], axis=0)
    return out.astype(np.float32), res


def kernel(x, Wq, Wk, Wv):
    out, _ = run_cores(np.asarray(x), np.asarray(Wq), np.asarray(Wk), np.asarray(Wv))
    return out
